# revision 1
# baseline (speedup 1.0000x reference)
"""Bahdanau attention scorer for Trainium2, 8-core data-parallel over batch.

scores[b, s] = v_a . tanh(W_s @ enc_outs[s, b] + W_t @ dec_out[b] + b_t)

Shapes (fixed): enc_outs (2048, 64, 512) f32, dec_out (64, 512) f32,
W_s/W_t (512, 512) f32, b_t/v_a (512,) f32 -> scores (64, 2048) f32.

Sharding: batch 64 -> 8 cores x 8. Small params replicated.

Per-core pipeline (all tokens = (b, s) pairs, BL=8 local batches):
  1. DMA enc shard tiles (128 s x 4096) HBM f32 -> SBUF bf16 (SWDGE cast).
  2. PE transposes 128x128 bf16 tiles -> PSUM -> DVE copy -> xT (h, s) SBUF.
  3. PE matmul psum[ac] (128 a x 512 s) += W_sT[hc,ac].T @ xT[hc]  (bf16).
  4. ACT: tanh(psum + bias[b, ac]) -> SBUF bf16   (bias = W_t@dec+b_t, per-part).
  5. PE matmul psumV (1, 512) += v_a[ac].T @ tanh[ac]  -> scores row.
"""

import sys

sys.path.insert(0, "/opt/trn_rl_repo")

import numpy as np
import ml_dtypes

import concourse.bass as bass
import concourse.mybir as mybir
import concourse.tile as tile
from concourse import bacc
from concourse.bass_utils import run_bass_kernel_spmd
from concourse.masks import make_identity

S, B, H, A = 2048, 64, 512, 512
NCORES = 8
BL = B // NCORES          # local batches per core
HC = H // 128             # h chunks
AC = A // 128             # a chunks
SBLK = 512                # s block (tokens per matmul group)
NSB = S // SBLK           # s blocks
ST = SBLK // 128          # 128-row s tiles per block

F32 = mybir.dt.float32
BF16 = mybir.dt.bfloat16
BF16_NP = ml_dtypes.bfloat16

_CACHE = {}


def build_kernel():
    nc = bacc.Bacc("TRN2", target_bir_lowering=False, debug=False,
                   num_devices=NCORES)

    enc_d = nc.dram_tensor("enc", [S, BL * H], F32, kind="ExternalInput")
    dec_d = nc.dram_tensor("dec", [BL, H], F32, kind="ExternalInput")
    wst_d = nc.dram_tensor("wst", [H, A], BF16, kind="ExternalInput")
    wtt_d = nc.dram_tensor("wtt", [H, A], BF16, kind="ExternalInput")
    bt4_d = nc.dram_tensor("bt4", [128, AC], F32, kind="ExternalInput")
    va4_d = nc.dram_tensor("va4", [128, AC], BF16, kind="ExternalInput")
    out_d = nc.dram_tensor("scores", [1, BL * S], F32, kind="ExternalOutput")

    with tile.TileContext(nc) as tc:
        with tc.tile_pool(name="consts", bufs=1) as constp:
            ident = constp.tile([128, 128], BF16, tag="ident")
            make_identity(nc, ident[:])

            wst_sb = constp.tile([128, HC * A], BF16, tag="wst")
            for hc in range(HC):
                nc.sync.dma_start(wst_sb[:, hc * A:(hc + 1) * A],
                                  wst_d[hc * 128:(hc + 1) * 128, :])
            va4_sb = constp.tile([128, AC], BF16, tag="va4")
            nc.sync.dma_start(va4_sb[:], va4_d[:])
            bt4_sb = constp.tile([128, AC], F32, tag="bt4")
            nc.sync.dma_start(bt4_sb[:], bt4_d[:])

            # ---- dec_att prep: bias[a, (ac, b)] = (W_t @ dec[b] + b_t)[a]
            bias_sb = constp.tile([128, AC * BL], F32, tag="bias")
            scores_sb = constp.tile([1, BL * S], F32, tag="scores")

            with (
                tc.tile_pool(name="prep", bufs=1) as prep,
                tc.tile_pool(name="prep_ps", bufs=1, space="PSUM") as prep_ps,
            ):
                wtt_sb = prep.tile([128, HC * A], BF16, tag="wtt")
                for hc in range(HC):
                    nc.sync.dma_start(wtt_sb[:, hc * A:(hc + 1) * A],
                                      wtt_d[hc * 128:(hc + 1) * 128, :])
                dec_sb = prep.tile([BL, H], BF16, tag="dec")
                nc.gpsimd.dma_start(dec_sb[:], dec_d[:])  # f32 -> bf16 cast

                # transpose dec (BL, H) -> decT (h, b) chunks
                pT0 = prep_ps.tile([128, HC * BL], BF16, tag="pT0")
                for hc in range(HC):
                    nc.tensor.matmul(
                        pT0[:, hc * BL:(hc + 1) * BL],
                        dec_sb[:, hc * 128:(hc + 1) * 128],
                        ident[0:BL, 0:BL],
                        is_transpose=True,
                        start=(hc == 0), stop=(hc == HC - 1),
                    )
                decT = prep.tile([128, HC * BL], BF16, tag="decT")
                nc.vector.tensor_copy(decT[:], pT0[:])

                for ac in range(AC):
                    ps_da = prep_ps.tile([128, BL], F32, tag=f"da{ac}")
                    for hc in range(HC):
                        nc.tensor.matmul(
                            ps_da[:],
                            wtt_sb[:, hc * A + ac * 128: hc * A + ac * 128 + 128],
                            decT[:, hc * BL:(hc + 1) * BL],
                            start=(hc == 0), stop=(hc == HC - 1),
                        )
                    nc.vector.tensor_scalar_add(
                        bias_sb[:, ac * BL:(ac + 1) * BL], ps_da[:],
                        bt4_sb[:, ac:ac + 1])

            # ---- main loop
            with (
                tc.tile_pool(name="xin", bufs=2 * ST) as loadp,
                tc.tile_pool(name="xt", bufs=2 * HC) as xtp,
                tc.tile_pool(name="tanh", bufs=2 * AC) as tanhp,
                tc.tile_pool(name="ps_t", bufs=2, space="PSUM") as pTp,
                tc.tile_pool(name="ps_mm", bufs=AC, space="PSUM") as mmp,
                tc.tile_pool(name="ps_v", bufs=2, space="PSUM") as pvp,
            ):
                for sb in range(NSB):
                    xin = []
                    for st in range(ST):
                        t = loadp.tile([128, BL * H], BF16, tag="xin")
                        r0 = (sb * ST + st) * 128
                        nc.gpsimd.dma_start(t[:], enc_d[r0:r0 + 128, :])
                        xin.append(t)

                    for b in range(BL):
                        # transpose: xT[hc] (128 h, SBLK s)
                        xT = []
                        for hc in range(HC):
                            pT = pTp.tile([128, SBLK], BF16, tag="pT")
                            for st in range(ST):
                                nc.tensor.matmul(
                                    pT[:, st * 128:(st + 1) * 128],
                                    xin[st][:, b * H + hc * 128:
                                            b * H + hc * 128 + 128],
                                    ident[:],
                                    is_transpose=True,
                                    start=(st == 0), stop=(st == ST - 1),
                                )
                            xt = xtp.tile([128, SBLK], BF16, tag="xt")
                            nc.vector.tensor_copy(xt[:], pT[:])
                            xT.append(xt)

                        # main matmul: psum[ac] (128 a, SBLK s)
                        psM = []
                        for ac in range(AC):
                            ps = mmp.tile([128, SBLK], F32, tag="mm")
                            for hc in range(HC):
                                nc.tensor.matmul(
                                    ps[:],
                                    wst_sb[:, hc * A + ac * 128:
                                           hc * A + ac * 128 + 128],
                                    xT[hc][:],
                                    start=(hc == 0), stop=(hc == HC - 1),
                                )
                            psM.append(ps)

                        # tanh(+bias) and v_a reduction
                        psV = pvp.tile([1, SBLK], F32, tag="pv")
                        for ac in range(AC):
                            th = tanhp.tile([128, SBLK], BF16, tag="tanh")
                            nc.scalar.activation(
                                th[:], psM[ac][:],
                                mybir.ActivationFunctionType.Tanh,
                                bias=bias_sb[:, ac * BL + b: ac * BL + b + 1],
                            )
                            nc.tensor.matmul(
                                psV[:], va4_sb[:, ac:ac + 1], th[:],
                                start=(ac == 0), stop=(ac == AC - 1),
                            )
                        nc.vector.tensor_copy(
                            scores_sb[0:1, b * S + sb * SBLK:
                                      b * S + (sb + 1) * SBLK],
                            psV[:])

            nc.sync.dma_start(out_d[:], scores_sb[:])

    nc.compile()
    return nc


def _prep_host(dec_out, enc_outs, W_s, W_t, b_t, v_a):
    wst = np.ascontiguousarray(W_s.T).astype(BF16_NP)
    wtt = np.ascontiguousarray(W_t.T).astype(BF16_NP)
    bt4 = np.ascontiguousarray(b_t.reshape(AC, 128).T).astype(np.float32)
    va4 = np.ascontiguousarray(v_a.reshape(AC, 128).T).astype(BF16_NP)
    in_maps = []
    for k in range(NCORES):
        enc = np.ascontiguousarray(
            enc_outs[:, k * BL:(k + 1) * BL, :]).reshape(S, BL * H)
        dec = np.ascontiguousarray(dec_out[k * BL:(k + 1) * BL, :])
        in_maps.append({
            "enc": enc.astype(np.float32),
            "dec": dec.astype(np.float32),
            "wst": wst, "wtt": wtt, "bt4": bt4, "va4": va4,
        })
    return in_maps


def kernel(dec_out, enc_outs, W_s, W_t, b_t, v_a, trace=False):
    dec_out = np.asarray(dec_out)
    enc_outs = np.asarray(enc_outs)
    if "nc" not in _CACHE:
        _CACHE["nc"] = build_kernel()
    nc = _CACHE["nc"]
    in_maps = _prep_host(dec_out, enc_outs,
                         np.asarray(W_s), np.asarray(W_t),
                         np.asarray(b_t), np.asarray(v_a))
    res = run_bass_kernel_spmd(nc, in_maps, core_ids=list(range(NCORES)),
                               trace=trace)
    out = np.concatenate(
        [res.results[k]["scores"].reshape(BL, S) for k in range(NCORES)],
        axis=0).astype(np.float32)
    if trace:
        _CACHE["last_result"] = res
    return out


# revision 5
# speedup vs baseline: 1.0667x; 1.0667x over previous
"""Bahdanau attention scorer for Trainium2, 8-core data-parallel over batch.

scores[b, s] = v_a . tanh(W_s @ enc_outs[s, b] + W_t @ dec_out[b] + b_t)

Shapes (fixed): enc_outs (2048, 64, 512) f32, dec_out (64, 512) f32,
W_s/W_t (512, 512) f32, b_t/v_a (512,) f32 -> scores (64, 2048) f32.

Sharding: batch 64 -> 8 cores x 8. Small params replicated.

Per-core pipeline (all tokens = (b, s) pairs, BL=8 local batches):
  1. DMA enc shard tiles (128 s x 4096) HBM f32 -> SBUF bf16 (SWDGE cast).
  2. PE transposes 128x128 bf16 tiles -> PSUM -> DVE copy -> xT (h, s) SBUF.
  3. PE matmul psum[ac] (128 a x 512 s) += W_sT[hc,ac].T @ xT[hc]  (bf16).
  4. ACT: tanh(psum + bias[b, ac]) -> SBUF bf16   (bias = W_t@dec+b_t, per-part).
  5. PE matmul psumV (1, 512) += v_a[ac].T @ tanh[ac]  -> scores row.
"""

import sys

sys.path.insert(0, "/opt/trn_rl_repo")

import numpy as np
import ml_dtypes

import concourse.bass as bass
import concourse.mybir as mybir
import concourse.tile as tile
from concourse import bacc
from concourse.bass_utils import run_bass_kernel_spmd
from concourse.masks import make_identity

S, B, H, A = 2048, 64, 512, 512
NCORES = 8
BL = B // NCORES          # local batches per core
HC = H // 128             # h chunks
AC = A // 128             # a chunks
SBLK = 512                # s block (tokens per matmul group)
NSB = S // SBLK           # s blocks
ST = SBLK // 128          # 128-row s tiles per block

F32 = mybir.dt.float32
BF16 = mybir.dt.bfloat16
BF16_NP = ml_dtypes.bfloat16

_CACHE = {}


def build_kernel():
    nc = bacc.Bacc("TRN2", target_bir_lowering=False, debug=False,
                   num_devices=NCORES)

    enc_d = nc.dram_tensor("enc", [S, BL * H], F32, kind="ExternalInput")
    dec_d = nc.dram_tensor("dec", [BL, H], F32, kind="ExternalInput")
    wst_d = nc.dram_tensor("wst", [H, A], BF16, kind="ExternalInput")
    wtt_d = nc.dram_tensor("wtt", [H, A], BF16, kind="ExternalInput")
    bt4_d = nc.dram_tensor("bt4", [128, AC], F32, kind="ExternalInput")
    va4_d = nc.dram_tensor("va4", [128, AC], BF16, kind="ExternalInput")
    out_d = nc.dram_tensor("scores", [1, BL * S], F32, kind="ExternalOutput")

    with tile.TileContext(nc) as tc:
        with tc.tile_pool(name="consts", bufs=1) as constp:
            ident = constp.tile([128, 128], BF16, tag="ident")
            make_identity(nc, ident[:])

            wst_sb = constp.tile([128, HC * A], BF16, tag="wst")
            for hc in range(HC):
                nc.sync.dma_start(wst_sb[:, hc * A:(hc + 1) * A],
                                  wst_d[hc * 128:(hc + 1) * 128, :])
            va4_sb = constp.tile([128, AC], BF16, tag="va4")
            nc.sync.dma_start(va4_sb[:], va4_d[:])
            bt4_sb = constp.tile([128, AC], F32, tag="bt4")
            nc.sync.dma_start(bt4_sb[:], bt4_d[:])

            # ---- dec_att prep: bias[a, (ac, b)] = (W_t @ dec[b] + b_t)[a]
            bias_sb = constp.tile([128, AC * BL], F32, tag="bias")
            scores_sb = constp.tile([1, BL * S], F32, tag="scores")

            with (
                tc.tile_pool(name="prep", bufs=1) as prep,
                tc.tile_pool(name="prep_ps", bufs=1, space="PSUM") as prep_ps,
            ):
                wtt_sb = prep.tile([128, HC * A], BF16, tag="wtt")
                for hc in range(HC):
                    nc.sync.dma_start(wtt_sb[:, hc * A:(hc + 1) * A],
                                      wtt_d[hc * 128:(hc + 1) * 128, :])
                dec_sb = prep.tile([BL, H], BF16, tag="dec")
                nc.gpsimd.dma_start(dec_sb[:], dec_d[:])  # f32 -> bf16 cast

                # transpose dec (BL, H) -> decT (h, b) chunks
                pT0 = prep_ps.tile([128, HC * BL], BF16, tag="pT0")
                for hc in range(HC):
                    nc.tensor.matmul(
                        pT0[:, hc * BL:(hc + 1) * BL],
                        dec_sb[:, hc * 128:(hc + 1) * 128],
                        ident[0:BL, 0:BL],
                        is_transpose=True,
                        start=(hc == 0), stop=(hc == HC - 1),
                    )
                decT = prep.tile([128, HC * BL], BF16, tag="decT")
                nc.vector.tensor_copy(decT[:], pT0[:])

                for ac in range(AC):
                    ps_da = prep_ps.tile([128, BL], F32, tag=f"da{ac}")
                    for hc in range(HC):
                        nc.tensor.matmul(
                            ps_da[:],
                            wtt_sb[:, hc * A + ac * 128: hc * A + ac * 128 + 128],
                            decT[:, hc * BL:(hc + 1) * BL],
                            start=(hc == 0), stop=(hc == HC - 1),
                        )
                    nc.vector.tensor_scalar_add(
                        bias_sb[:, ac * BL:(ac + 1) * BL], ps_da[:],
                        bt4_sb[:, ac:ac + 1])

            # ---- main loop
            with (
                tc.tile_pool(name="xin", bufs=2 * ST) as loadp,
                tc.tile_pool(name="xt", bufs=2 * HC) as xtp,
                tc.tile_pool(name="tanh", bufs=2 * AC) as tanhp,
                tc.tile_pool(name="ps_t", bufs=2, space="PSUM") as pTp,
                tc.tile_pool(name="ps_mm", bufs=AC, space="PSUM") as mmp,
                tc.tile_pool(name="ps_v", bufs=2, space="PSUM") as pvp,
            ):
                HB = BL // 2 * H  # free width of a half tile (b 0-3 | 4-7)
                for sb in range(NSB):
                    xin = [[None, None] for _ in range(ST)]  # [st][half]
                    for h2 in range(2):
                        for st in range(ST):
                            r0 = (sb * ST + st) * 128
                            t = loadp.tile([128, HB], BF16, tag=f"xin{h2}")
                            nc.gpsimd.dma_start(
                                t[:], enc_d[r0:r0 + 128, h2 * HB:(h2 + 1) * HB])
                            xin[st][h2] = t

                    for b in range(BL):
                        h2, bh = divmod(b, BL // 2)
                        # transpose: xT[hc] (128 h, SBLK s)
                        xT = []
                        for hc in range(HC):
                            pT = pTp.tile([128, SBLK], BF16, tag="pT")
                            for st in range(ST):
                                nc.tensor.matmul(
                                    pT[:, st * 128:(st + 1) * 128],
                                    xin[st][h2][:, bh * H + hc * 128:
                                                bh * H + hc * 128 + 128],
                                    ident[:],
                                    is_transpose=True,
                                    start=(st == 0), stop=(st == ST - 1),
                                )
                            xt = xtp.tile([128, SBLK], BF16, tag="xt")
                            nc.vector.tensor_copy(xt[:], pT[:])
                            xT.append(xt)

                        # main matmul: psum[ac] (128 a, SBLK s)
                        psM = []
                        for ac in range(AC):
                            ps = mmp.tile([128, SBLK], F32, tag="mm")
                            for hc in range(HC):
                                nc.tensor.matmul(
                                    ps[:],
                                    wst_sb[:, hc * A + ac * 128:
                                           hc * A + ac * 128 + 128],
                                    xT[hc][:],
                                    start=(hc == 0), stop=(hc == HC - 1),
                                )
                            psM.append(ps)

                        # tanh(+bias) and v_a reduction
                        psV = pvp.tile([1, SBLK], F32, tag="pv")
                        for ac in range(AC):
                            th = tanhp.tile([128, SBLK], BF16, tag="tanh")
                            nc.scalar.activation(
                                th[:], psM[ac][:],
                                mybir.ActivationFunctionType.Tanh,
                                bias=bias_sb[:, ac * BL + b: ac * BL + b + 1],
                            )
                            nc.tensor.matmul(
                                psV[:], va4_sb[:, ac:ac + 1], th[:],
                                start=(ac == 0), stop=(ac == AC - 1),
                            )
                        nc.vector.tensor_copy(
                            scores_sb[0:1, b * S + sb * SBLK:
                                      b * S + (sb + 1) * SBLK],
                            psV[:])
                        nc.sync.dma_start(
                            out_d[0:1, b * S + sb * SBLK:
                                  b * S + (sb + 1) * SBLK],
                            scores_sb[0:1, b * S + sb * SBLK:
                                      b * S + (sb + 1) * SBLK])

    nc.compile()
    return nc


def _prep_host(dec_out, enc_outs, W_s, W_t, b_t, v_a):
    wst = np.ascontiguousarray(W_s.T).astype(BF16_NP)
    wtt = np.ascontiguousarray(W_t.T).astype(BF16_NP)
    bt4 = np.ascontiguousarray(b_t.reshape(AC, 128).T).astype(np.float32)
    va4 = np.ascontiguousarray(v_a.reshape(AC, 128).T).astype(BF16_NP)
    in_maps = []
    for k in range(NCORES):
        enc = np.ascontiguousarray(
            enc_outs[:, k * BL:(k + 1) * BL, :]).reshape(S, BL * H)
        dec = np.ascontiguousarray(dec_out[k * BL:(k + 1) * BL, :])
        in_maps.append({
            "enc": enc.astype(np.float32),
            "dec": dec.astype(np.float32),
            "wst": wst, "wtt": wtt, "bt4": bt4, "va4": va4,
        })
    return in_maps


def kernel(dec_out, enc_outs, W_s, W_t, b_t, v_a, trace=False):
    dec_out = np.asarray(dec_out)
    enc_outs = np.asarray(enc_outs)
    if "nc" not in _CACHE:
        _CACHE["nc"] = build_kernel()
    nc = _CACHE["nc"]
    in_maps = _prep_host(dec_out, enc_outs,
                         np.asarray(W_s), np.asarray(W_t),
                         np.asarray(b_t), np.asarray(v_a))
    res = run_bass_kernel_spmd(nc, in_maps, core_ids=list(range(NCORES)),
                               trace=trace)
    out = np.concatenate(
        [res.results[k]["scores"].reshape(BL, S) for k in range(NCORES)],
        axis=0).astype(np.float32)
    if trace:
        _CACHE["last_result"] = res
    return out


# revision 8
# speedup vs baseline: 1.1441x; 1.0725x over previous
"""Bahdanau attention scorer for Trainium2, 8-core data-parallel over batch.

scores[b, s] = v_a . tanh(W_s @ enc_outs[s, b] + W_t @ dec_out[b] + b_t)

Shapes (fixed): enc_outs (2048, 64, 512) f32, dec_out (64, 512) f32,
W_s/W_t (512, 512) f32, b_t/v_a (512,) f32 -> scores (64, 2048) f32.

Sharding: batch 64 -> 8 cores x 8. Small params replicated.

Per-core pipeline (all tokens = (b, s) pairs, BL=8 local batches):
  1. DMA enc shard tiles (128 s x 4096) HBM f32 -> SBUF bf16 (SWDGE cast).
  2. PE transposes 128x128 bf16 tiles -> PSUM -> DVE copy -> xT (h, s) SBUF.
  3. PE matmul psum[ac] (128 a x 512 s) += W_sT[hc,ac].T @ xT[hc]  (bf16).
  4. ACT: tanh(psum + bias[b, ac]) -> SBUF bf16   (bias = W_t@dec+b_t, per-part).
  5. PE matmul psumV (1, 512) += v_a[ac].T @ tanh[ac]  -> scores row.
"""

import sys

sys.path.insert(0, "/opt/trn_rl_repo")

import numpy as np
import ml_dtypes

import concourse.bass as bass
import concourse.mybir as mybir
import concourse.tile as tile
from concourse import bacc
from concourse.bass_utils import run_bass_kernel_spmd
from concourse.masks import make_identity

S, B, H, A = 2048, 64, 512, 512
NCORES = 8
BL = B // NCORES          # local batches per core
HC = H // 128             # h chunks
AC = A // 128             # a chunks
SBLK = 512                # s block (tokens per matmul group)
NSB = S // SBLK           # s blocks
ST = SBLK // 128          # 128-row s tiles per block

F32 = mybir.dt.float32
BF16 = mybir.dt.bfloat16
BF16_NP = ml_dtypes.bfloat16

_CACHE = {}


def build_kernel():
    nc = bacc.Bacc("TRN2", target_bir_lowering=False, debug=False,
                   num_devices=NCORES)

    enc_d = nc.dram_tensor("enc", [S, BL * H], F32, kind="ExternalInput")
    dec_d = nc.dram_tensor("dec", [BL, H], F32, kind="ExternalInput")
    wst_d = nc.dram_tensor("wst", [H, A], BF16, kind="ExternalInput")
    wtt_d = nc.dram_tensor("wtt", [H, A], BF16, kind="ExternalInput")
    bt4_d = nc.dram_tensor("bt4", [128, AC], F32, kind="ExternalInput")
    va4_d = nc.dram_tensor("va4", [128, AC], F32, kind="ExternalInput")
    out_d = nc.dram_tensor("scores", [1, BL * S], F32, kind="ExternalOutput")

    with tile.TileContext(nc) as tc:
        with tc.tile_pool(name="consts", bufs=1) as constp:
            ident = constp.tile([128, 128], BF16, tag="ident")
            make_identity(nc, ident[:])

            wst_sb = constp.tile([128, HC * A], BF16, tag="wst")
            for hc in range(HC):
                nc.sync.dma_start(wst_sb[:, hc * A:(hc + 1) * A],
                                  wst_d[hc * 128:(hc + 1) * 128, :])
            ones_sb = constp.tile([128, 1], BF16, tag="ones")
            nc.gpsimd.memset(ones_sb[:], 1.0)
            va4_sb = constp.tile([128, AC], F32, tag="va4")
            nc.sync.dma_start(va4_sb[:], va4_d[:])
            bt4_sb = constp.tile([128, AC], F32, tag="bt4")
            nc.sync.dma_start(bt4_sb[:], bt4_d[:])

            # ---- dec_att prep: bias[a, (ac, b)] = (W_t @ dec[b] + b_t)[a]
            bias_sb = constp.tile([128, AC * BL], F32, tag="bias")
            scores_sb = constp.tile([1, BL * S], F32, tag="scores")

            with (
                tc.tile_pool(name="prep", bufs=1) as prep,
                tc.tile_pool(name="prep_ps", bufs=1, space="PSUM") as prep_ps,
            ):
                wtt_sb = prep.tile([128, HC * A], BF16, tag="wtt")
                for hc in range(HC):
                    nc.sync.dma_start(wtt_sb[:, hc * A:(hc + 1) * A],
                                      wtt_d[hc * 128:(hc + 1) * 128, :])
                dec_sb = prep.tile([BL, H], BF16, tag="dec")
                nc.gpsimd.dma_start(dec_sb[:], dec_d[:])  # f32 -> bf16 cast

                # transpose dec (BL, H) -> decT (h, b) chunks
                pT0 = prep_ps.tile([128, HC * BL], BF16, tag="pT0")
                for hc in range(HC):
                    nc.tensor.matmul(
                        pT0[:, hc * BL:(hc + 1) * BL],
                        dec_sb[:, hc * 128:(hc + 1) * 128],
                        ident[0:BL, 0:BL],
                        is_transpose=True,
                        start=(hc == 0), stop=(hc == HC - 1),
                    )
                decT = prep.tile([128, HC * BL], BF16, tag="decT")
                nc.vector.tensor_copy(decT[:], pT0[:])

                for ac in range(AC):
                    ps_da = prep_ps.tile([128, BL], F32, tag=f"da{ac}")
                    for hc in range(HC):
                        nc.tensor.matmul(
                            ps_da[:],
                            wtt_sb[:, hc * A + ac * 128: hc * A + ac * 128 + 128],
                            decT[:, hc * BL:(hc + 1) * BL],
                            start=(hc == 0), stop=(hc == HC - 1),
                        )
                    nc.vector.tensor_scalar_add(
                        bias_sb[:, ac * BL:(ac + 1) * BL], ps_da[:],
                        bt4_sb[:, ac:ac + 1])

            # ---- main loop
            with (
                tc.tile_pool(name="xin", bufs=2 * ST) as loadp,
                tc.tile_pool(name="xt", bufs=2 * HC) as xtp,
                tc.tile_pool(name="tanh", bufs=2 * AC) as tanhp,
                tc.tile_pool(name="ps_t", bufs=2, space="PSUM") as pTp,
                tc.tile_pool(name="ps_mm", bufs=AC, space="PSUM") as mmp,
                tc.tile_pool(name="ps_v", bufs=2, space="PSUM") as pvp,
            ):
                HB = BL // 2 * H  # free width of a half tile (b 0-3 | 4-7)
                for sb in range(NSB):
                    xin = [[None, None] for _ in range(ST)]  # [st][half]
                    for h2 in range(2):
                        for st in range(ST):
                            r0 = (sb * ST + st) * 128
                            t = loadp.tile([128, HB], BF16, tag=f"xin{h2}")
                            nc.gpsimd.dma_start(
                                t[:], enc_d[r0:r0 + 128, h2 * HB:(h2 + 1) * HB])
                            xin[st][h2] = t

                    for b in range(BL):
                        h2, bh = divmod(b, BL // 2)
                        # transpose: xT[hc] (128 h, SBLK s)
                        xT = []
                        for hc in range(HC):
                            pT = pTp.tile([128, SBLK], BF16, tag="pT")
                            for st in range(ST):
                                nc.tensor.matmul(
                                    pT[:, st * 128:(st + 1) * 128],
                                    xin[st][h2][:, bh * H + hc * 128:
                                                bh * H + hc * 128 + 128],
                                    ident[:],
                                    is_transpose=True,
                                    start=(st == 0), stop=(st == ST - 1),
                                )
                            xt = xtp.tile([128, SBLK], BF16, tag="xt")
                            nc.vector.tensor_copy(xt[:], pT[:])
                            xT.append(xt)

                        # main matmul: psum[ac] (128 a, SBLK s)
                        psM = []
                        for ac in range(AC):
                            ps = mmp.tile([128, SBLK], F32, tag="mm")
                            for hc in range(HC):
                                nc.tensor.matmul(
                                    ps[:],
                                    wst_sb[:, hc * A + ac * 128:
                                           hc * A + ac * 128 + 128],
                                    xT[hc][:],
                                    start=(hc == 0), stop=(hc == HC - 1),
                                )
                            psM.append(ps)

                        # tanh(+bias), v_a weighting on DVE, ones-reduce on PE
                        psV = pvp.tile([1, SBLK], F32, tag="pv")
                        ths = []
                        for ac in range(AC):
                            th = tanhp.tile([128, SBLK], BF16, tag="tanh")
                            nc.scalar.activation(
                                th[:], psM[ac][:],
                                mybir.ActivationFunctionType.Tanh,
                                bias=bias_sb[:, ac * BL + b: ac * BL + b + 1],
                            )
                            vm = tanhp.tile([128, SBLK], BF16, tag="vm")
                            nc.vector.tensor_scalar_mul(
                                vm[:], th[:], va4_sb[:, ac:ac + 1])
                            ths.append(vm)
                        nc.vector.tensor_add(ths[0][:], ths[0][:], ths[1][:])
                        nc.vector.tensor_add(ths[2][:], ths[2][:], ths[3][:])
                        nc.vector.tensor_add(ths[0][:], ths[0][:], ths[2][:])
                        nc.tensor.matmul(psV[:], ones_sb[:], ths[0][:],
                                         start=True, stop=True)
                        nc.vector.tensor_copy(
                            scores_sb[0:1, b * S + sb * SBLK:
                                      b * S + (sb + 1) * SBLK],
                            psV[:])
                        nc.sync.dma_start(
                            out_d[0:1, b * S + sb * SBLK:
                                  b * S + (sb + 1) * SBLK],
                            scores_sb[0:1, b * S + sb * SBLK:
                                      b * S + (sb + 1) * SBLK])

    nc.compile()
    return nc


def _prep_host(dec_out, enc_outs, W_s, W_t, b_t, v_a):
    wst = np.ascontiguousarray(W_s.T).astype(BF16_NP)
    wtt = np.ascontiguousarray(W_t.T).astype(BF16_NP)
    bt4 = np.ascontiguousarray(b_t.reshape(AC, 128).T).astype(np.float32)
    va4 = np.ascontiguousarray(v_a.reshape(AC, 128).T).astype(np.float32)
    in_maps = []
    for k in range(NCORES):
        enc = np.ascontiguousarray(
            enc_outs[:, k * BL:(k + 1) * BL, :]).reshape(S, BL * H)
        dec = np.ascontiguousarray(dec_out[k * BL:(k + 1) * BL, :])
        in_maps.append({
            "enc": enc.astype(np.float32),
            "dec": dec.astype(np.float32),
            "wst": wst, "wtt": wtt, "bt4": bt4, "va4": va4,
        })
    return in_maps


def kernel(dec_out, enc_outs, W_s, W_t, b_t, v_a, trace=False):
    dec_out = np.asarray(dec_out)
    enc_outs = np.asarray(enc_outs)
    if "nc" not in _CACHE:
        _CACHE["nc"] = build_kernel()
    nc = _CACHE["nc"]
    in_maps = _prep_host(dec_out, enc_outs,
                         np.asarray(W_s), np.asarray(W_t),
                         np.asarray(b_t), np.asarray(v_a))
    res = run_bass_kernel_spmd(nc, in_maps, core_ids=list(range(NCORES)),
                               trace=trace)
    out = np.concatenate(
        [res.results[k]["scores"].reshape(BL, S) for k in range(NCORES)],
        axis=0).astype(np.float32)
    if trace:
        _CACHE["last_result"] = res
    return out
